# revision 13
# baseline (speedup 1.0000x reference)
"""Trainium2 Bass kernel for nn_BothGuideAttentionLayer.

Reference computation (per branch, branch params (D, S)):
    q,k,v = x@W* + b*                          [B,S,D]
    A = softmax(band_mask(S, w=2))             [S,S]  constant, row-stochastic
    kk = A @ k
    P = softmax(q @ kk^T / sqrt(D))
    attn = P @ v
    y1 = attn + x ; h1 = whole_norm(y1) (GLOBAL mean/var over [B,S,D]) * nw + nb
    f  = relu(h1@W1 + b1)@W2 + b2
    y2 = f + x ; out = whole_norm(y2) * nw + nb

Sharding: data-parallel over batch (8 batches -> 8 cores). The two global
norms need cross-core sums -> 4 tiny AllReduces (sum, sumsq per branch per
norm), each overlapped with the other branch's compute.

Layout: all activations are kept TRANSPOSED [D, S] (D on partitions) so that
every matmul consumes weights in natural [K, M] layout.  k, v are produced in
[S, D] (needed as stationary operands of the attention matmuls).  The host
pre-transposes x/nw/nb per batch and precomputes A^T.  Bias handling uses the
row-stochasticity of A and P: bk and bv commute through the attention matmuls
and are applied as per-partition ACT biases on kk^T and attn^T.

Image branch: S=196 padded to 256 (clean 128-partition chunks and N>=256 for
fp32r); score pad columns are masked to -1e30 before softmax and all norm
statistics are computed on sliced [:, :196] views.
"""

import math
import os
import sys

import numpy as np

for _p in ("/opt/trn_rl_repo",):
    if _p not in sys.path and os.path.isdir(_p):
        sys.path.insert(0, _p)

import concourse.bass as bass
import concourse.tile as tile
from concourse import bacc, mybir
from concourse.bass_utils import run_bass_kernel_spmd
from concourse.masks import make_identity

F32 = mybir.dt.float32
AX = mybir.AxisListType.X
AF = mybir.ActivationFunctionType
ALU = mybir.AluOpType

N_CORES = 8
B = 8
WINDOW = 2
EPS = 1e-6

# Matmul operand dtype: "f32" (exact, 4 cyc/row) or "f32r" (1 cyc/row at
# N>=256, reduced internal precision).  Overridable via env for experiments.
MM_DTYPE = os.environ.get("BASS_MM_DTYPE", "f32")
MMF = mybir.dt.float32r if MM_DTYPE == "f32r" else mybir.dt.float32

TEXT = dict(pre="t", D=1024, S=512, Sr=512, H=4096)
IMG = dict(pre="i", D=768, S=256, Sr=196, H=3072)


def _n_chunks(total, step=512):
    out = []
    off = 0
    while off < total:
        sz = min(step, total - off)
        out.append((off, sz))
        off += sz
    return out


def make_A(S):
    idx = np.arange(S)
    mask = (np.abs(idx[:, None] - idx[None, :]) <= WINDOW).astype(np.float32)
    e = np.exp(mask - mask.max(axis=-1, keepdims=True))
    return e / e.sum(axis=-1, keepdims=True)




class Branch:
    """Holds per-branch DRAM handles and SBUF state during build."""

    def __init__(self, nc, p):
        self.p = p
        pre = p["pre"]
        D, S, H = p["D"], p["S"], p["H"]
        self.DC, self.SC, self.HC = D // 128, S // 128, H // 128
        t = nc.dram_tensor
        self.xT = t(f"{pre}_xT", [D, S], MMF, kind="ExternalInput")
        self.Wq = t(f"{pre}_Wq", [D, D], MMF, kind="ExternalInput")
        self.Wk = t(f"{pre}_Wk", [D, D], MMF, kind="ExternalInput")
        self.Wv = t(f"{pre}_Wv", [D, D], MMF, kind="ExternalInput")
        self.W1 = t(f"{pre}_W1", [D, H], MMF, kind="ExternalInput")
        self.W2 = t(f"{pre}_W2", [H, D], MMF, kind="ExternalInput")
        self.bqs = t(f"{pre}_bqs", [128, self.DC], F32, kind="ExternalInput")
        self.bk = t(f"{pre}_bk", [128, self.DC], F32, kind="ExternalInput")
        self.bv = t(f"{pre}_bv", [128, self.DC], F32, kind="ExternalInput")
        self.b1 = t(f"{pre}_b1", [128, self.HC], F32, kind="ExternalInput")
        self.b2 = t(f"{pre}_b2", [128, self.DC], F32, kind="ExternalInput")
        self.AT = t(f"{pre}_AT", [S, S], MMF, kind="ExternalInput")
        self.nwT = t(f"{pre}_nwT", [D, S], F32, kind="ExternalInput")
        self.nbT = t(f"{pre}_nbT", [D, S], F32, kind="ExternalInput")
        self.outT = t(f"{pre}_outT", [D, S], F32, kind="ExternalOutput")
        # number of elements of the FULL (all-batch) tensor for the mean
        self.Nfull = float(B * p["Sr"] * D)


def build_kernel():
    nc = bacc.Bacc(
        "TRN2",
        target_bir_lowering=False,
        debug=False,
        enable_asserts=False,
        num_devices=N_CORES,
    )
    br_t = Branch(nc, TEXT)
    br_i = Branch(nc, IMG)

    with tile.TileContext(nc, pool_alloc_mode="queue") as tc:
        _build(tc, br_t, br_i)
    nc.compile()
    return nc


def _build(tc, br_t, br_i):
    nc = tc.nc

    cpool = tc.alloc_tile_pool(name="cpool", bufs=1)
    psum = tc.alloc_tile_pool(name="psum", bufs=1, space="PSUM")
    dram = tc.alloc_tile_pool(name="dram", bufs=1, space="DRAM")

    ident = cpool.tile([128, 128], F32, name="ident")
    make_identity(nc, ident)
    ones_col = cpool.tile([128, 1], F32, name="ones_col")
    nc.vector.memset(ones_col, 1.0)
    eps_col = cpool.tile([128, 1], F32, name="eps_col")
    nc.vector.memset(eps_col, EPS)

    class C:
        pass

    g = C()
    g.psum = psum
    g.dram = dram
    g.ident = ident
    g.ones = ones_col
    g.eps = eps_col

    for br in (br_t, br_i):
        pre = br.p["pre"]
        # constants in SBUF for the whole kernel
        br.at_sb = cpool.tile([128, br.SC, br.p["S"]], MMF, name=f"{pre}_at_sb")
        nc.sync.dma_start(
            br.at_sb, br.AT.ap().rearrange("(c q) s -> q c s", q=128)
        )
        for nm in ("bqs", "bk", "bv", "b1", "b2"):
            t = cpool.tile(
                [128, br.HC if nm == "b1" else br.DC], F32, name=f"{pre}_{nm}_sb"
            )
            nc.sync.dma_start(t, getattr(br, nm).ap())
            setattr(br, nm + "_sb", t)

    # ---- schedule: interleave branches around the 4 AllReduces ----
    # persistent pools first so the scratch pools rotate in one contiguous
    # ring region after them
    br_t.persist = tc.alloc_tile_pool(name="t_persist", bufs=1)
    br_i.persist = tc.alloc_tile_pool(name="i_persist", bufs=1)

    br_t.attn_pool = tc.alloc_tile_pool(name="t_attn", bufs=1)
    _attention_phase(tc, g, br_t)
    br_t.attn_pool.release()
    _start_allreduce(tc, g, br_t, "n1")

    br_i.attn_pool = tc.alloc_tile_pool(name="i_attn", bufs=1)
    _attention_phase(tc, g, br_i)
    br_i.attn_pool.release()
    _start_allreduce(tc, g, br_i, "n1")

    br_t.ffn_pool = tc.alloc_tile_pool(name="t_ffn", bufs=1)
    _ffn_phase(tc, g, br_t)
    br_t.ffn_pool.release()
    _start_allreduce(tc, g, br_t, "n2")

    br_i.ffn_pool = tc.alloc_tile_pool(name="i_ffn", bufs=1)
    _ffn_phase(tc, g, br_i)
    br_i.ffn_pool.release()
    _start_allreduce(tc, g, br_i, "n2")

    br_t.fin_pool = tc.alloc_tile_pool(name="t_fin", bufs=1)
    _final_phase(tc, g, br_t)
    br_t.fin_pool.release()

    br_i.fin_pool = tc.alloc_tile_pool(name="i_fin", bufs=1)
    _final_phase(tc, g, br_i)
    br_i.fin_pool.release()

    br_i.persist.release()
    br_t.persist.release()
    dram.release()
    psum.release()
    cpool.release()


def _matmul_acc(nc, ps, lhs_tiles, rhs_tiles):
    n = len(lhs_tiles)
    for k in range(n):
        nc.tensor.matmul(
            ps,
            lhs_tiles[k],
            rhs_tiles[k],
            start=(k == 0),
            stop=(k == n - 1),
        )


def _attention_phase(tc, g, br):
    nc = tc.nc
    p = br.p
    pre = p["pre"]
    D, S, Sr, DC, SC = p["D"], p["S"], p["Sr"], br.DC, br.SC
    sp = br.attn_pool
    pp = br.persist

    # persistent activations
    br.xT_sb = pp.tile([128, DC, S], MMF, name=f"{pre}_xT_sb")
    nc.sync.dma_start(br.xT_sb, br.xT.ap().rearrange("(c q) s -> q c s", q=128))
    # y1T and y2T share one slot: y1T is dead once h1T is computed.
    br.y1T = pp.tile([128, DC, S], F32, tag="ybuf", name=f"{pre}_y1T")

    qTs = sp.tile([128, DC, S], MMF, name=f"{pre}_qTs")
    k_sb = sp.tile([128, SC, D], MMF, name=f"{pre}_k_sb")
    v_sb = sp.tile([128, SC, D], MMF, name=f"{pre}_v_sb")

    wq_dram = br.Wq.ap().rearrange("(c q) m -> q c m", q=128)
    # --- qT[m] = (Wq[:,m])^T @ xT, scaled by 1/sqrt(D), bias bq/sqrt(D) ---
    inv_sqrt_d = 1.0 / math.sqrt(D)
    for m in range(DC):
        wqm = sp.tile([128, DC, 128], MMF, tag="wq", bufs=3, name="wqm")
        nc.sync.dma_start(wqm, wq_dram[:, :, m * 128 : (m + 1) * 128])
        ps = g.psum.tile([128, S], F32, tag="mm", bufs=4, name="ps_q")
        _matmul_acc(
            nc,
            ps,
            [wqm[:, k, :] for k in range(DC)],
            [br.xT_sb[:, k, :] for k in range(DC)],
        )
        nc.scalar.activation(
            qTs[:, m, :],
            ps,
            AF.Identity,
            bias=br.bqs_sb[:, m : m + 1],
            scale=inv_sqrt_d,
        )

    # --- k, v in [S, D] (no bias; folded downstream) ---
    for W, out_sb in ((br.Wk, k_sb), (br.Wv, v_sb)):
        wd = W.ap()
        for noff, nsz in _n_chunks(D):
            wtiles = []
            for k in range(DC):
                wt = sp.tile([128, 512], MMF, tag="wkv", bufs=DC + 4, name="wt")
                nc.sync.dma_start(
                    wt[:, :nsz], wd[k * 128 : (k + 1) * 128, noff : noff + nsz]
                )
                wtiles.append(wt)
            for sc in range(SC):
                ps = g.psum.tile([128, nsz], F32, tag="mm", bufs=4, name="ps_kv")
                _matmul_acc(
                    nc,
                    ps,
                    [br.xT_sb[:, k, sc * 128 : (sc + 1) * 128] for k in range(DC)],
                    [wtiles[k][:, :nsz] for k in range(DC)],
                )
                nc.vector.tensor_copy(out_sb[:, sc, noff : noff + nsz], ps)

    # --- kkT[m] = k^T @ A^T + bk ---
    kkT = sp.tile([128, DC, S], MMF, name=f"{pre}_kkT")
    for m in range(DC):
        ps = g.psum.tile([128, S], F32, tag="mm", bufs=4, name="ps_kk")
        _matmul_acc(
            nc,
            ps,
            [k_sb[:, sc, m * 128 : (m + 1) * 128] for sc in range(SC)],
            [br.at_sb[:, sc, :] for sc in range(SC)],
        )
        nc.scalar.activation(
            kkT[:, m, :], ps, AF.Identity, bias=br.bk_sb[:, m : m + 1]
        )

    # --- scores + softmax -> P ---
    P_sb = sp.tile([128, SC, S], F32, name=f"{pre}_P")
    for sc in range(SC):
        ps = g.psum.tile([128, S], F32, tag="mm", bufs=4, name="ps_sc")
        _matmul_acc(
            nc,
            ps,
            [qTs[:, m, sc * 128 : (sc + 1) * 128] for m in range(DC)],
            [kkT[:, m, :] for m in range(DC)],
        )
        if Sr < S:
            nc.vector.memset(ps[:, Sr:S], -1e30)
        nmax = sp.tile([128, 1], F32, tag="nmax", bufs=4, name="nmax")
        nc.vector.reduce_max(nmax, ps, AX, negate=True)
        rsum = sp.tile([128, 1], F32, tag="rsum", bufs=4, name="rsum")
        nc.scalar.activation(
            P_sb[:, sc, :], ps, AF.Exp, bias=nmax, scale=1.0, accum_out=rsum
        )
        rcp = sp.tile([128, 1], F32, tag="rcp", bufs=4, name="rcp")
        nc.vector.reciprocal(rcp, rsum)
        nc.vector.tensor_scalar_mul(P_sb[:, sc, :], P_sb[:, sc, :], rcp)

    # --- PT via PE transpose ---
    PT_sb = sp.tile([128, SC, S], MMF, name=f"{pre}_PT")
    for j in range(SC):
        for sc in range(SC):
            tp = g.psum.tile([128, 128], F32, tag="tp", bufs=2, name="tp")
            nc.tensor.transpose(tp, P_sb[:, sc, j * 128 : (j + 1) * 128], g.ident)
            nc.vector.tensor_copy(PT_sb[:, j, sc * 128 : (sc + 1) * 128], tp)

    # --- attnT + bv + residual -> y1T ; stats ---
    sum_cols = pp.tile([128, DC], F32, name=f"{pre}_sum1")
    ssq_cols = pp.tile([128, DC], F32, name=f"{pre}_ssq1")
    for m in range(DC):
        ps = g.psum.tile([128, S], F32, tag="mm", bufs=4, name="ps_at")
        _matmul_acc(
            nc,
            ps,
            [v_sb[:, j, m * 128 : (m + 1) * 128] for j in range(SC)],
            [PT_sb[:, j, :] for j in range(SC)],
        )
        nc.vector.scalar_tensor_tensor(
            out=br.y1T[:, m, :],
            in0=ps,
            scalar=br.bv_sb[:, m : m + 1],
            in1=br.xT_sb[:, m, :],
            op0=ALU.add,
            op1=ALU.add,
        )
        nc.vector.reduce_sum(sum_cols[:, m : m + 1], br.y1T[:, m, :Sr], AX)
        sq = sp.tile([128, S], F32, tag="sq", bufs=2, name="sq")
        nc.scalar.activation(
            sq[:, :Sr],
            br.y1T[:, m, :Sr],
            AF.Square,
            accum_out=ssq_cols[:, m : m + 1],
        )
    _emit_stats(tc, g, br, sum_cols, ssq_cols, "n1")


def _emit_stats(tc, g, br, sum_cols, ssq_cols, tag):
    """Partition-reduce [128,DC] sums into a [1,2] pair staged for AllReduce."""
    nc = tc.nc
    pre = br.p["pre"]
    pair = br.persist.tile([128, 2], F32, name=f"{pre}_{tag}_pair")
    nc.vector.reduce_sum(pair[:, 0:1], sum_cols, AX)
    nc.vector.reduce_sum(pair[:, 1:2], ssq_cols, AX)
    ps = g.psum.tile([1, 2], F32, tag="st", bufs=1, name="ps_st")
    nc.tensor.matmul(ps, g.ones, pair, start=True, stop=True)
    stage = br.persist.tile([1, 8], F32, name=f"{pre}_{tag}_stage")
    nc.vector.memset(stage, 0.0)
    nc.vector.tensor_copy(stage[:, 0:2], ps)
    setattr(br, f"ar_stage_{tag}", stage)


def _start_allreduce(tc, g, br, tag):
    nc = tc.nc
    pre = br.p["pre"]
    stage = getattr(br, f"ar_stage_{tag}")
    ar_in = g.dram.tile([1, 8], F32, name=f"{pre}_{tag}_arin")
    ar_out = g.dram.tile([1, 8], F32, name=f"{pre}_{tag}_arout")
    nc.sync.dma_start(ar_in, stage)
    nc.gpsimd.collective_compute(
        "AllReduce",
        ALU.add,
        replica_groups=[list(range(N_CORES))],
        ins=[ar_in.opt()],
        outs=[ar_out.opt()],
    )
    bc = br.persist.tile([128, 2], F32, name=f"{pre}_{tag}_bc")
    nc.sync.dma_start(bc, ar_out[0:1, 0:2].to_broadcast([128, 2]))
    setattr(br, f"ar_bc_{tag}", bc)


def _norm_coefs(tc, g, br, tag):
    """From broadcast [128,2] (sum, sumsq) compute a = rsqrt(var+eps),
    c = -mu * a as per-partition [128,1] columns."""
    nc = tc.nc
    pre = br.p["pre"]
    pool = br.ffn_pool if tag == "n1" else br.fin_pool
    bc = getattr(br, f"ar_bc_{tag}")
    inv_n = 1.0 / br.Nfull
    mu = pool.tile([128, 1], F32, name=f"{pre}_{tag}_mu")
    nc.vector.tensor_scalar_mul(mu, bc[:, 0:1], inv_n)
    e2 = pool.tile([128, 1], F32, name=f"{pre}_{tag}_e2")
    nc.vector.tensor_scalar_mul(e2, bc[:, 1:2], inv_n)
    var = pool.tile([128, 1], F32, name=f"{pre}_{tag}_var")
    nc.vector.tensor_mul(var, mu, mu)
    nc.vector.tensor_sub(var, e2, var)
    sd = pool.tile([128, 1], F32, name=f"{pre}_{tag}_sd")
    nc.scalar.activation(sd, var, AF.Sqrt, bias=g.eps[0:128, :])
    a = pool.tile([128, 1], F32, name=f"{pre}_{tag}_a")
    nc.vector.reciprocal(a, sd)
    c = pool.tile([128, 1], F32, name=f"{pre}_{tag}_c")
    nc.vector.tensor_mul(c, mu, a)
    nc.vector.tensor_scalar_mul(c, c, -1.0)
    return a, c


def _apply_norm(tc, g, br, pool, src_tile_fn, a, c, out_fn):
    """out[m] = (src[m]*a + c) * nwT[m] + nbT[m] for each m chunk."""
    nc = tc.nc
    p = br.p
    D, S, DC = p["D"], p["S"], br.DC
    nw_dram = br.nwT.ap().rearrange("(c q) s -> q c s", q=128)
    nb_dram = br.nbT.ap().rearrange("(c q) s -> q c s", q=128)
    for m in range(DC):
        nw = pool.tile([128, S], F32, tag="nw", bufs=3, name="nw")
        nc.sync.dma_start(nw, nw_dram[:, m, :])
        nb = pool.tile([128, S], F32, tag="nb", bufs=3, name="nb")
        nc.sync.dma_start(nb, nb_dram[:, m, :])
        t1 = pool.tile([128, S], F32, tag="normt1", bufs=3, name="t1")
        nc.scalar.activation(t1, src_tile_fn(m), AF.Identity, bias=c, scale=a)
        nc.vector.tensor_mul(t1, t1, nw)
        out_fn(m, t1, nb)


def _ffn_phase(tc, g, br):
    nc = tc.nc
    p = br.p
    pre = p["pre"]
    D, S, Sr, H, DC, SC, HC = p["D"], p["S"], p["Sr"], p["H"], br.DC, br.SC, br.HC
    fp = br.ffn_pool

    a1, c1 = _norm_coefs(tc, g, br, "n1")
    h1T = fp.tile([128, DC, S], MMF, name=f"{pre}_h1T")

    def h1_out(m, t1, nb):
        nc.vector.tensor_add(h1T[:, m, :], t1, nb)

    _apply_norm(tc, g, br, fp, lambda m: br.y1T[:, m, :], a1, c1, h1_out)

    # --- zT[h] = relu(W1[:,h]^T @ h1T + b1[h]) ---
    zT = fp.tile([128, HC, S], MMF, name=f"{pre}_zT")
    w1_dram = br.W1.ap().rearrange("(c q) m -> q c m", q=128)
    for h in range(HC):
        w1m = fp.tile([128, DC, 128], MMF, tag="w1", bufs=3, name="w1m")
        nc.sync.dma_start(w1m, w1_dram[:, :, h * 128 : (h + 1) * 128])
        ps = g.psum.tile([128, S], F32, tag="mm", bufs=4, name="ps_z")
        _matmul_acc(
            nc,
            ps,
            [w1m[:, k, :] for k in range(DC)],
            [h1T[:, k, :] for k in range(DC)],
        )
        nc.scalar.activation(
            zT[:, h, :], ps, AF.Relu, bias=br.b1_sb[:, h : h + 1]
        )

    # --- y2T[m] = W2[:,m]^T @ zT + b2[m] + xT[m] ; stats ---
    br.y2T = br.persist.tile([128, DC, S], F32, tag="ybuf", name=f"{pre}_y2T")
    sum_cols = br.persist.tile([128, DC], F32, name=f"{pre}_sum2")
    ssq_cols = br.persist.tile([128, DC], F32, name=f"{pre}_ssq2")
    w2_dram = br.W2.ap().rearrange("(c q) m -> q c m", q=128)
    HH = HC // 2
    for m in range(DC):
        w2a = fp.tile([128, HH, 128], MMF, tag="w2", bufs=3, name="w2a")
        nc.sync.dma_start(w2a, w2_dram[:, :HH, m * 128 : (m + 1) * 128])
        w2b = fp.tile([128, HH, 128], MMF, tag="w2", bufs=3, name="w2b")
        nc.sync.dma_start(w2b, w2_dram[:, HH:, m * 128 : (m + 1) * 128])
        ps = g.psum.tile([128, S], F32, tag="mm", bufs=4, name="ps_f")
        _matmul_acc(
            nc,
            ps,
            [w2a[:, h, :] for h in range(HH)]
            + [w2b[:, h, :] for h in range(HH)],
            [zT[:, h, :] for h in range(HC)],
        )
        nc.vector.scalar_tensor_tensor(
            out=br.y2T[:, m, :],
            in0=ps,
            scalar=br.b2_sb[:, m : m + 1],
            in1=br.xT_sb[:, m, :],
            op0=ALU.add,
            op1=ALU.add,
        )
        nc.vector.reduce_sum(sum_cols[:, m : m + 1], br.y2T[:, m, :Sr], AX)
        sq = fp.tile([128, S], F32, tag="sq2", bufs=2, name="sq2")
        nc.scalar.activation(
            sq[:, :Sr],
            br.y2T[:, m, :Sr],
            AF.Square,
            accum_out=ssq_cols[:, m : m + 1],
        )
    _emit_stats(tc, g, br, sum_cols, ssq_cols, "n2")


def _final_phase(tc, g, br):
    nc = tc.nc
    p = br.p
    DC = br.DC
    a2, c2 = _norm_coefs(tc, g, br, "n2")
    out_dram = br.outT.ap().rearrange("(c q) s -> q c s", q=128)

    def fin_out(m, t1, nb):
        nc.vector.tensor_add(t1, t1, nb)
        nc.sync.dma_start(out_dram[:, m, :], t1)

    _apply_norm(tc, g, br, br.fin_pool, lambda m: br.y2T[:, m, :], a2, c2, fin_out)


# ----------------------------------------------------------------------------
# host side
# ----------------------------------------------------------------------------

_CACHE = {}

# test-harness knobs (harmless defaults for grading)
TRACE = False
TRACE_KWARGS = {}
LAST_RESULT = None


def _get_compiled():
    key = MM_DTYPE
    if key not in _CACHE:
        _CACHE[key] = build_kernel()
    return _CACHE[key]


def _prep_branch_inputs(pre, x, Wq, bq, Wk, bk, Wv, bv, W1, b1, W2, b2, nw, nb, p):
    D, S, Sr, H = p["D"], p["S"], p["Sr"], p["H"]
    DC, HC = D // 128, H // 128
    f = np.float32

    def col(v, C):
        return np.ascontiguousarray(np.asarray(v, f).reshape(C, 128).T)

    A = make_A(Sr)
    AT = np.zeros((S, S), f)
    AT[:Sr, :Sr] = A.T

    shared = {
        f"{pre}_Wq": np.ascontiguousarray(np.asarray(Wq, f)),
        f"{pre}_Wk": np.ascontiguousarray(np.asarray(Wk, f)),
        f"{pre}_Wv": np.ascontiguousarray(np.asarray(Wv, f)),
        f"{pre}_W1": np.ascontiguousarray(np.asarray(W1, f)),
        f"{pre}_W2": np.ascontiguousarray(np.asarray(W2, f)),
        f"{pre}_bqs": col(np.asarray(bq, f) / np.sqrt(D).astype(f), DC),
        f"{pre}_bk": col(bk, DC),
        f"{pre}_bv": col(bv, DC),
        f"{pre}_b1": col(b1, HC),
        f"{pre}_b2": col(b2, DC),
        f"{pre}_AT": AT,
    }
    per_core = []
    x = np.asarray(x, f)
    nw = np.asarray(nw, f)
    nb = np.asarray(nb, f)
    for b in range(B):
        xT = np.zeros((D, S), f)
        xT[:, :Sr] = x[b].T
        nwT = np.zeros((D, S), f)
        nwT[:, :Sr] = nw[b].T
        nbT = np.zeros((D, S), f)
        nbT[:, :Sr] = nb[b].T
        per_core.append(
            {f"{pre}_xT": xT, f"{pre}_nwT": nwT, f"{pre}_nbT": nbT}
        )
    return shared, per_core


def kernel(**inputs):
    nc = _get_compiled()

    sh_t, pc_t = _prep_branch_inputs(
        "t",
        inputs["text_feature"],
        inputs["t_Wq"], inputs["t_bq"], inputs["t_Wk"], inputs["t_bk"],
        inputs["t_Wv"], inputs["t_bv"], inputs["t_W1"], inputs["t_b1"],
        inputs["t_W2"], inputs["t_b2"], inputs["t_nw"], inputs["t_nb"],
        TEXT,
    )
    sh_i, pc_i = _prep_branch_inputs(
        "i",
        inputs["image_feature"],
        inputs["i_Wq"], inputs["i_bq"], inputs["i_Wk"], inputs["i_bk"],
        inputs["i_Wv"], inputs["i_bv"], inputs["i_W1"], inputs["i_b1"],
        inputs["i_W2"], inputs["i_b2"], inputs["i_nw"], inputs["i_nb"],
        IMG,
    )

    in_maps = []
    for c in range(N_CORES):
        m = {}
        m.update(sh_t)
        m.update(sh_i)
        m.update(pc_t[c])
        m.update(pc_i[c])
        in_maps.append(m)

    global LAST_RESULT
    res = run_bass_kernel_spmd(
        nc, in_maps, core_ids=list(range(N_CORES)), trace=TRACE, **TRACE_KWARGS
    )
    LAST_RESULT = res
    outs = res.results

    f = np.float32
    text_out = np.stack(
        [np.asarray(outs[c]["t_outT"], f)[:, : TEXT["Sr"]].T for c in range(B)]
    )
    image_out = np.stack(
        [np.asarray(outs[c]["i_outT"], f)[:, : IMG["Sr"]].T for c in range(B)]
    )
    return (text_out, image_out)


if __name__ == "__main__":
    nc = build_kernel()
    counts = {}
    for bb in nc.main_func.blocks:
        for ins in bb.instructions:
            counts[type(ins).__name__] = counts.get(type(ins).__name__, 0) + 1
    print(counts)


# revision 15
# speedup vs baseline: 1.2142x; 1.2142x over previous
"""Trainium2 Bass kernel for nn_BothGuideAttentionLayer.

Reference computation (per branch, branch params (D, S)):
    q,k,v = x@W* + b*                          [B,S,D]
    A = softmax(band_mask(S, w=2))             [S,S]  constant, row-stochastic
    kk = A @ k
    P = softmax(q @ kk^T / sqrt(D))
    attn = P @ v
    y1 = attn + x ; h1 = whole_norm(y1) (GLOBAL mean/var over [B,S,D]) * nw + nb
    f  = relu(h1@W1 + b1)@W2 + b2
    y2 = f + x ; out = whole_norm(y2) * nw + nb

Sharding: data-parallel over batch (8 batches -> 8 cores). The two global
norms need cross-core sums -> 4 tiny AllReduces (sum, sumsq per branch per
norm), each overlapped with the other branch's compute.

Layout: all activations are kept TRANSPOSED [D, S] (D on partitions) so that
every matmul consumes weights in natural [K, M] layout.  k, v are produced in
[S, D] (needed as stationary operands of the attention matmuls).  The host
pre-transposes x/nw/nb per batch and precomputes A^T.  Bias handling uses the
row-stochasticity of A and P: bk and bv commute through the attention matmuls
and are applied as per-partition ACT biases on kk^T and attn^T.

Image branch: S=196 padded to 256 (clean 128-partition chunks and N>=256 for
fp32r); score pad columns are masked to -1e30 before softmax and all norm
statistics are computed on sliced [:, :196] views.
"""

import math
import os
import sys

import numpy as np

for _p in ("/opt/trn_rl_repo",):
    if _p not in sys.path and os.path.isdir(_p):
        sys.path.insert(0, _p)

import concourse.bass as bass
import concourse.tile as tile
from concourse import bacc, mybir
from concourse.bass_utils import run_bass_kernel_spmd
from concourse.masks import make_identity

F32 = mybir.dt.float32
AX = mybir.AxisListType.X
AF = mybir.ActivationFunctionType
ALU = mybir.AluOpType

N_CORES = 8
B = 8
WINDOW = 2
EPS = 1e-6

# Matmul operand dtype: "f32" (exact, 4 cyc/row) or "f32r" (1 cyc/row at
# N>=256, reduced internal precision).  Overridable via env for experiments.
MM_DTYPE = os.environ.get("BASS_MM_DTYPE", "f32r")
MMF = {
    "f32": mybir.dt.float32,
    "f32r": mybir.dt.float32r,
    "bf16": mybir.dt.bfloat16,
}[MM_DTYPE]

TEXT = dict(pre="t", D=1024, S=512, Sr=512, H=4096)
IMG = dict(pre="i", D=768, S=256, Sr=196, H=3072)


def _n_chunks(total, step=512):
    out = []
    off = 0
    while off < total:
        sz = min(step, total - off)
        out.append((off, sz))
        off += sz
    return out


def make_A(S):
    idx = np.arange(S)
    mask = (np.abs(idx[:, None] - idx[None, :]) <= WINDOW).astype(np.float32)
    e = np.exp(mask - mask.max(axis=-1, keepdims=True))
    return e / e.sum(axis=-1, keepdims=True)




class Branch:
    """Holds per-branch DRAM handles and SBUF state during build."""

    def __init__(self, nc, p):
        self.p = p
        pre = p["pre"]
        D, S, H = p["D"], p["S"], p["H"]
        self.DC, self.SC, self.HC = D // 128, S // 128, H // 128
        t = nc.dram_tensor
        self.xT = t(f"{pre}_xT", [D, S], MMF, kind="ExternalInput")
        if MM_DTYPE == "bf16":
            self.xR = t(f"{pre}_xR", [D, S], F32, kind="ExternalInput")
        self.Wq = t(f"{pre}_Wq", [D, D], MMF, kind="ExternalInput")
        self.Wk = t(f"{pre}_Wk", [D, D], MMF, kind="ExternalInput")
        self.Wv = t(f"{pre}_Wv", [D, D], MMF, kind="ExternalInput")
        self.W1 = t(f"{pre}_W1", [D, H], MMF, kind="ExternalInput")
        self.W2 = t(f"{pre}_W2", [H, D], MMF, kind="ExternalInput")
        self.bqs = t(f"{pre}_bqs", [128, self.DC], F32, kind="ExternalInput")
        self.bk = t(f"{pre}_bk", [128, self.DC], F32, kind="ExternalInput")
        self.bv = t(f"{pre}_bv", [128, self.DC], F32, kind="ExternalInput")
        self.b1 = t(f"{pre}_b1", [128, self.HC], F32, kind="ExternalInput")
        self.b2 = t(f"{pre}_b2", [128, self.DC], F32, kind="ExternalInput")
        self.AT = t(f"{pre}_AT", [S, S], MMF, kind="ExternalInput")
        self.nwT = t(f"{pre}_nwT", [D, S], F32, kind="ExternalInput")
        self.nbT = t(f"{pre}_nbT", [D, S], F32, kind="ExternalInput")
        self.outT = t(f"{pre}_outT", [D, S], F32, kind="ExternalOutput")
        # number of elements of the FULL (all-batch) tensor for the mean
        self.Nfull = float(B * p["Sr"] * D)


def build_kernel():
    nc = bacc.Bacc(
        "TRN2",
        target_bir_lowering=False,
        debug=False,
        enable_asserts=False,
        num_devices=N_CORES,
    )
    br_t = Branch(nc, TEXT)
    br_i = Branch(nc, IMG)

    with tile.TileContext(nc, pool_alloc_mode="queue") as tc:
        _build(tc, br_t, br_i)
    nc.compile()
    return nc


def _build(tc, br_t, br_i):
    nc = tc.nc

    cpool = tc.alloc_tile_pool(name="cpool", bufs=1)
    psum = tc.alloc_tile_pool(name="psum", bufs=1, space="PSUM")
    dram = tc.alloc_tile_pool(name="dram", bufs=1, space="DRAM")

    ident = cpool.tile([128, 128], F32, name="ident")
    make_identity(nc, ident)
    ones_col = cpool.tile([128, 1], F32, name="ones_col")
    nc.vector.memset(ones_col, 1.0)
    eps_col = cpool.tile([128, 1], F32, name="eps_col")
    nc.vector.memset(eps_col, EPS)

    class C:
        pass

    g = C()
    g.psum = psum
    g.dram = dram
    g.ident = ident
    g.ones = ones_col
    g.eps = eps_col

    for br in (br_t, br_i):
        br.cpool = cpool

    # ---- schedule: interleave branches around the 4 AllReduces ----
    # persistent pools first so the scratch pools rotate in one contiguous
    # ring region after them
    br_t.persist = tc.alloc_tile_pool(name="t_persist", bufs=1)
    br_i.persist = tc.alloc_tile_pool(name="i_persist", bufs=1)

    br_t.attn_pool = tc.alloc_tile_pool(name="t_attn", bufs=1)
    _attention_phase(tc, g, br_t)
    br_t.attn_pool.release()
    _start_allreduce(tc, g, br_t, "n1")

    br_i.attn_pool = tc.alloc_tile_pool(name="i_attn", bufs=1)
    _attention_phase(tc, g, br_i)
    br_i.attn_pool.release()
    _start_allreduce(tc, g, br_i, "n1")

    br_t.ffn_pool = tc.alloc_tile_pool(name="t_ffn", bufs=1)
    _ffn_phase(tc, g, br_t)
    br_t.ffn_pool.release()
    _start_allreduce(tc, g, br_t, "n2")

    br_i.ffn_pool = tc.alloc_tile_pool(name="i_ffn", bufs=1)
    _ffn_phase(tc, g, br_i)
    br_i.ffn_pool.release()
    _start_allreduce(tc, g, br_i, "n2")

    br_t.fin_pool = tc.alloc_tile_pool(name="t_fin", bufs=1)
    _final_phase(tc, g, br_t)
    br_t.fin_pool.release()

    br_i.fin_pool = tc.alloc_tile_pool(name="i_fin", bufs=1)
    _final_phase(tc, g, br_i)
    br_i.fin_pool.release()

    br_i.persist.release()
    br_t.persist.release()
    dram.release()
    psum.release()
    cpool.release()


def _matmul_acc(nc, ps, lhs_tiles, rhs_tiles):
    n = len(lhs_tiles)
    for k in range(n):
        nc.tensor.matmul(
            ps,
            lhs_tiles[k],
            rhs_tiles[k],
            start=(k == 0),
            stop=(k == n - 1),
        )


def _attention_phase(tc, g, br):
    nc = tc.nc
    p = br.p
    pre = p["pre"]
    D, S, Sr, DC, SC = p["D"], p["S"], p["Sr"], br.DC, br.SC
    sp = br.attn_pool
    pp = br.persist

    # persistent activations (x + first weight DMAs first: PE startup path)
    br.xT_sb = pp.tile([128, DC, S], MMF, name=f"{pre}_xT_sb")
    nc.sync.dma_start(br.xT_sb, br.xT.ap().rearrange("(c q) s -> q c s", q=128))
    if MM_DTYPE == "bf16":
        br.xR_sb = pp.tile([128, DC, S], F32, name=f"{pre}_xR_sb")
        nc.sync.dma_start(
            br.xR_sb, br.xR.ap().rearrange("(c q) s -> q c s", q=128)
        )
    else:
        br.xR_sb = br.xT_sb
    # y1T and y2T share one slot: y1T is dead once h1T is computed.
    br.y1T = pp.tile([128, DC, S], F32, tag="ybuf", name=f"{pre}_y1T")

    qTs = sp.tile([128, DC, S], MMF, name=f"{pre}_qTs")
    k_sb = sp.tile([128, SC, D], MMF, name=f"{pre}_k_sb")
    v_sb = sp.tile([128, SC, D], MMF, name=f"{pre}_v_sb")

    # branch constants; emitted after xT so the first weight DMAs aren't
    # stuck behind them in the DMA queue
    cp = br.cpool
    for nm in ("bqs", "bk", "bv", "b1", "b2"):
        t = cp.tile([128, br.HC if nm == "b1" else br.DC], F32, name=f"{pre}_{nm}_sb")
        nc.sync.dma_start(t, getattr(br, nm).ap())
        setattr(br, nm + "_sb", t)
    br.at_sb = cp.tile([128, SC, S], MMF, name=f"{pre}_at_sb")
    nc.sync.dma_start(br.at_sb, br.AT.ap().rearrange("(c q) s -> q c s", q=128))

    wq_dram = br.Wq.ap().rearrange("(c q) m -> q c m", q=128)
    # --- qT[m] = (Wq[:,m])^T @ xT, scaled by 1/sqrt(D), bias bq/sqrt(D) ---
    inv_sqrt_d = 1.0 / math.sqrt(D)
    for m in range(DC):
        wqm = sp.tile([128, DC, 128], MMF, tag="wq", bufs=3, name="wqm")
        nc.sync.dma_start(wqm, wq_dram[:, :, m * 128 : (m + 1) * 128])
        ps = g.psum.tile([128, S], F32, tag="mm", bufs=4, name="ps_q")
        _matmul_acc(
            nc,
            ps,
            [wqm[:, k, :] for k in range(DC)],
            [br.xT_sb[:, k, :] for k in range(DC)],
        )
        nc.scalar.activation(
            qTs[:, m, :],
            ps,
            AF.Identity,
            bias=br.bqs_sb[:, m : m + 1],
            scale=inv_sqrt_d,
        )

    # --- k, v in [S, D] (no bias; folded downstream) ---
    for W, out_sb in ((br.Wk, k_sb), (br.Wv, v_sb)):
        wd = W.ap()
        for noff, nsz in _n_chunks(D):
            wtiles = []
            for k in range(DC):
                wt = sp.tile([128, 512], MMF, tag="wkv", bufs=DC + 4, name="wt")
                nc.sync.dma_start(
                    wt[:, :nsz], wd[k * 128 : (k + 1) * 128, noff : noff + nsz]
                )
                wtiles.append(wt)
            for sc in range(SC):
                ps = g.psum.tile([128, nsz], F32, tag="mm", bufs=4, name="ps_kv")
                _matmul_acc(
                    nc,
                    ps,
                    [br.xT_sb[:, k, sc * 128 : (sc + 1) * 128] for k in range(DC)],
                    [wtiles[k][:, :nsz] for k in range(DC)],
                )
                nc.vector.tensor_copy(out_sb[:, sc, noff : noff + nsz], ps)

    # --- kkT[m] = k^T @ A^T + bk ---
    kkT = sp.tile([128, DC, S], MMF, name=f"{pre}_kkT")
    for m in range(DC):
        ps = g.psum.tile([128, S], F32, tag="mm", bufs=4, name="ps_kk")
        _matmul_acc(
            nc,
            ps,
            [k_sb[:, sc, m * 128 : (m + 1) * 128] for sc in range(SC)],
            [br.at_sb[:, sc, :] for sc in range(SC)],
        )
        nc.scalar.activation(
            kkT[:, m, :], ps, AF.Identity, bias=br.bk_sb[:, m : m + 1]
        )

    # --- scores + softmax -> P ---
    P_sb = sp.tile([128, SC, S], F32, name=f"{pre}_P")
    for sc in range(SC):
        ps = g.psum.tile([128, S], F32, tag="mm", bufs=4, name="ps_sc")
        _matmul_acc(
            nc,
            ps,
            [qTs[:, m, sc * 128 : (sc + 1) * 128] for m in range(DC)],
            [kkT[:, m, :] for m in range(DC)],
        )
        if Sr < S:
            nc.vector.memset(ps[:, Sr:S], -1e30)
        nmax = sp.tile([128, 1], F32, tag="nmax", bufs=4, name="nmax")
        nc.vector.reduce_max(nmax, ps, AX, negate=True)
        rsum = sp.tile([128, 1], F32, tag="rsum", bufs=4, name="rsum")
        nc.scalar.activation(
            P_sb[:, sc, :], ps, AF.Exp, bias=nmax, scale=1.0, accum_out=rsum
        )
        rcp = sp.tile([128, 1], F32, tag="rcp", bufs=4, name="rcp")
        nc.vector.reciprocal(rcp, rsum)
        nc.vector.tensor_scalar_mul(P_sb[:, sc, :], P_sb[:, sc, :], rcp)

    # --- PT via PE transpose ---
    PT_sb = sp.tile([128, SC, S], MMF, name=f"{pre}_PT")
    for j in range(SC):
        for sc in range(SC):
            tp = g.psum.tile([128, 128], F32, tag="tp", bufs=2, name="tp")
            nc.tensor.transpose(tp, P_sb[:, sc, j * 128 : (j + 1) * 128], g.ident)
            nc.vector.tensor_copy(PT_sb[:, j, sc * 128 : (sc + 1) * 128], tp)

    # --- attnT + bv + residual -> y1T ; stats ---
    sum_cols = pp.tile([128, DC], F32, name=f"{pre}_sum1")
    ssq_cols = pp.tile([128, DC], F32, name=f"{pre}_ssq1")
    for m in range(DC):
        ps = g.psum.tile([128, S], F32, tag="mm", bufs=4, name="ps_at")
        _matmul_acc(
            nc,
            ps,
            [v_sb[:, j, m * 128 : (m + 1) * 128] for j in range(SC)],
            [PT_sb[:, j, :] for j in range(SC)],
        )
        nc.vector.scalar_tensor_tensor(
            out=br.y1T[:, m, :],
            in0=ps,
            scalar=br.bv_sb[:, m : m + 1],
            in1=br.xR_sb[:, m, :],
            op0=ALU.add,
            op1=ALU.add,
        )
        nc.vector.reduce_sum(sum_cols[:, m : m + 1], br.y1T[:, m, :Sr], AX)
        sq = sp.tile([128, S], F32, tag="sq", bufs=2, name="sq")
        nc.scalar.activation(
            sq[:, :Sr],
            br.y1T[:, m, :Sr],
            AF.Square,
            accum_out=ssq_cols[:, m : m + 1],
        )
    _emit_stats(tc, g, br, sum_cols, ssq_cols, "n1")


def _emit_stats(tc, g, br, sum_cols, ssq_cols, tag):
    """Partition-reduce [128,DC] sums into a [1,2] pair staged for AllReduce."""
    nc = tc.nc
    pre = br.p["pre"]
    pair = br.persist.tile([128, 2], F32, name=f"{pre}_{tag}_pair")
    nc.vector.reduce_sum(pair[:, 0:1], sum_cols, AX)
    nc.vector.reduce_sum(pair[:, 1:2], ssq_cols, AX)
    ps = g.psum.tile([1, 2], F32, tag="st", bufs=1, name="ps_st")
    nc.tensor.matmul(ps, g.ones, pair, start=True, stop=True)
    stage = br.persist.tile([1, 8], F32, name=f"{pre}_{tag}_stage")
    nc.vector.memset(stage, 0.0)
    nc.vector.tensor_copy(stage[:, 0:2], ps)
    setattr(br, f"ar_stage_{tag}", stage)


def _start_allreduce(tc, g, br, tag):
    nc = tc.nc
    pre = br.p["pre"]
    stage = getattr(br, f"ar_stage_{tag}")
    ar_in = g.dram.tile([1, 8], F32, name=f"{pre}_{tag}_arin")
    ar_out = g.dram.tile([1, 8], F32, name=f"{pre}_{tag}_arout")
    nc.sync.dma_start(ar_in, stage)
    nc.gpsimd.collective_compute(
        "AllReduce",
        ALU.add,
        replica_groups=[list(range(N_CORES))],
        ins=[ar_in.opt()],
        outs=[ar_out.opt()],
    )
    bc = br.persist.tile([128, 2], F32, name=f"{pre}_{tag}_bc")
    nc.sync.dma_start(bc, ar_out[0:1, 0:2].to_broadcast([128, 2]))
    setattr(br, f"ar_bc_{tag}", bc)


def _norm_coefs(tc, g, br, tag):
    """From broadcast [128,2] (sum, sumsq) compute a = rsqrt(var+eps),
    c = -mu * a as per-partition [128,1] columns."""
    nc = tc.nc
    pre = br.p["pre"]
    pool = br.ffn_pool if tag == "n1" else br.fin_pool
    bc = getattr(br, f"ar_bc_{tag}")
    inv_n = 1.0 / br.Nfull
    mu = pool.tile([128, 1], F32, name=f"{pre}_{tag}_mu")
    nc.vector.tensor_scalar_mul(mu, bc[:, 0:1], inv_n)
    e2 = pool.tile([128, 1], F32, name=f"{pre}_{tag}_e2")
    nc.vector.tensor_scalar_mul(e2, bc[:, 1:2], inv_n)
    var = pool.tile([128, 1], F32, name=f"{pre}_{tag}_var")
    nc.vector.tensor_mul(var, mu, mu)
    nc.vector.tensor_sub(var, e2, var)
    sd = pool.tile([128, 1], F32, name=f"{pre}_{tag}_sd")
    nc.scalar.activation(sd, var, AF.Sqrt, bias=g.eps[0:128, :])
    a = pool.tile([128, 1], F32, name=f"{pre}_{tag}_a")
    nc.vector.reciprocal(a, sd)
    c = pool.tile([128, 1], F32, name=f"{pre}_{tag}_c")
    nc.vector.tensor_mul(c, mu, a)
    nc.vector.tensor_scalar_mul(c, c, -1.0)
    return a, c


def _apply_norm(tc, g, br, pool, src_tile_fn, a, c, out_fn):
    """out[m] = (src[m]*a + c) * nwT[m] + nbT[m] for each m chunk."""
    nc = tc.nc
    p = br.p
    D, S, DC = p["D"], p["S"], br.DC
    nw_dram = br.nwT.ap().rearrange("(c q) s -> q c s", q=128)
    nb_dram = br.nbT.ap().rearrange("(c q) s -> q c s", q=128)
    for m in range(DC):
        nw = pool.tile([128, S], F32, tag="nw", bufs=3, name="nw")
        nc.sync.dma_start(nw, nw_dram[:, m, :])
        nb = pool.tile([128, S], F32, tag="nb", bufs=3, name="nb")
        nc.sync.dma_start(nb, nb_dram[:, m, :])
        t1 = pool.tile([128, S], F32, tag="normt1", bufs=3, name="t1")
        nc.scalar.activation(t1, src_tile_fn(m), AF.Identity, bias=c, scale=a)
        nc.vector.tensor_mul(t1, t1, nw)
        out_fn(m, t1, nb)


def _ffn_phase(tc, g, br):
    nc = tc.nc
    p = br.p
    pre = p["pre"]
    D, S, Sr, H, DC, SC, HC = p["D"], p["S"], p["Sr"], p["H"], br.DC, br.SC, br.HC
    fp = br.ffn_pool

    w1_dram_pre = br.W1.ap().rearrange("(c q) m -> q c m", q=128)
    w1_pre = []
    for h in range(min(3, HC)):
        w1m = fp.tile([128, DC, 128], MMF, tag="w1", bufs=3, name="w1m")
        nc.sync.dma_start(w1m, w1_dram_pre[:, :, h * 128 : (h + 1) * 128])
        w1_pre.append(w1m)

    a1, c1 = _norm_coefs(tc, g, br, "n1")
    h1T = fp.tile([128, DC, S], MMF, name=f"{pre}_h1T")

    def h1_out(m, t1, nb):
        nc.vector.tensor_add(h1T[:, m, :], t1, nb)

    _apply_norm(tc, g, br, fp, lambda m: br.y1T[:, m, :], a1, c1, h1_out)

    # --- zT[h] = relu(W1[:,h]^T @ h1T + b1[h]) ---
    zT = fp.tile([128, HC, S], MMF, name=f"{pre}_zT")
    w1_dram = br.W1.ap().rearrange("(c q) m -> q c m", q=128)
    for h in range(HC):
        if h < len(w1_pre):
            w1m = w1_pre[h]
        else:
            w1m = fp.tile([128, DC, 128], MMF, tag="w1", bufs=3, name="w1m")
            nc.sync.dma_start(w1m, w1_dram[:, :, h * 128 : (h + 1) * 128])
        ps = g.psum.tile([128, S], F32, tag="mm", bufs=4, name="ps_z")
        _matmul_acc(
            nc,
            ps,
            [w1m[:, k, :] for k in range(DC)],
            [h1T[:, k, :] for k in range(DC)],
        )
        nc.scalar.activation(
            zT[:, h, :], ps, AF.Relu, bias=br.b1_sb[:, h : h + 1]
        )

    # --- y2T[m] = W2[:,m]^T @ zT + b2[m] + xT[m] ; stats ---
    br.y2T = br.persist.tile([128, DC, S], F32, tag="ybuf", name=f"{pre}_y2T")
    sum_cols = br.persist.tile([128, DC], F32, name=f"{pre}_sum2")
    ssq_cols = br.persist.tile([128, DC], F32, name=f"{pre}_ssq2")
    w2_dram = br.W2.ap().rearrange("(c q) m -> q c m", q=128)
    HH = HC // 2
    for m in range(DC):
        w2a = fp.tile([128, HH, 128], MMF, tag="w2", bufs=3, name="w2a")
        nc.sync.dma_start(w2a, w2_dram[:, :HH, m * 128 : (m + 1) * 128])
        w2b = fp.tile([128, HH, 128], MMF, tag="w2", bufs=3, name="w2b")
        nc.sync.dma_start(w2b, w2_dram[:, HH:, m * 128 : (m + 1) * 128])
        ps = g.psum.tile([128, S], F32, tag="mm", bufs=4, name="ps_f")
        _matmul_acc(
            nc,
            ps,
            [w2a[:, h, :] for h in range(HH)]
            + [w2b[:, h, :] for h in range(HH)],
            [zT[:, h, :] for h in range(HC)],
        )
        nc.vector.scalar_tensor_tensor(
            out=br.y2T[:, m, :],
            in0=ps,
            scalar=br.b2_sb[:, m : m + 1],
            in1=br.xR_sb[:, m, :],
            op0=ALU.add,
            op1=ALU.add,
        )
        nc.vector.reduce_sum(sum_cols[:, m : m + 1], br.y2T[:, m, :Sr], AX)
        sq = fp.tile([128, S], F32, tag="sq2", bufs=2, name="sq2")
        nc.scalar.activation(
            sq[:, :Sr],
            br.y2T[:, m, :Sr],
            AF.Square,
            accum_out=ssq_cols[:, m : m + 1],
        )
    _emit_stats(tc, g, br, sum_cols, ssq_cols, "n2")


def _final_phase(tc, g, br):
    nc = tc.nc
    p = br.p
    DC = br.DC
    a2, c2 = _norm_coefs(tc, g, br, "n2")
    out_dram = br.outT.ap().rearrange("(c q) s -> q c s", q=128)

    def fin_out(m, t1, nb):
        nc.vector.tensor_add(t1, t1, nb)
        nc.sync.dma_start(out_dram[:, m, :], t1)

    _apply_norm(tc, g, br, br.fin_pool, lambda m: br.y2T[:, m, :], a2, c2, fin_out)


# ----------------------------------------------------------------------------
# host side
# ----------------------------------------------------------------------------

_CACHE = {}

# test-harness knobs (harmless defaults for grading)
TRACE = False
TRACE_KWARGS = {}
LAST_RESULT = None


def _get_compiled():
    key = MM_DTYPE
    if key not in _CACHE:
        _CACHE[key] = build_kernel()
    return _CACHE[key]


def _prep_branch_inputs(pre, x, Wq, bq, Wk, bk, Wv, bv, W1, b1, W2, b2, nw, nb, p):
    D, S, Sr, H = p["D"], p["S"], p["Sr"], p["H"]
    DC, HC = D // 128, H // 128
    f = np.float32

    def col(v, C):
        return np.ascontiguousarray(np.asarray(v, f).reshape(C, 128).T)

    A = make_A(Sr)
    AT = np.zeros((S, S), f)
    AT[:Sr, :Sr] = A.T

    if MM_DTYPE == "bf16":
        import ml_dtypes

        mf = ml_dtypes.bfloat16
    else:
        mf = f

    shared = {
        f"{pre}_Wq": np.ascontiguousarray(np.asarray(Wq, f).astype(mf)),
        f"{pre}_Wk": np.ascontiguousarray(np.asarray(Wk, f).astype(mf)),
        f"{pre}_Wv": np.ascontiguousarray(np.asarray(Wv, f).astype(mf)),
        f"{pre}_W1": np.ascontiguousarray(np.asarray(W1, f).astype(mf)),
        f"{pre}_W2": np.ascontiguousarray(np.asarray(W2, f).astype(mf)),
        f"{pre}_bqs": col(np.asarray(bq, f) / np.sqrt(D).astype(f), DC),
        f"{pre}_bk": col(bk, DC),
        f"{pre}_bv": col(bv, DC),
        f"{pre}_b1": col(b1, HC),
        f"{pre}_b2": col(b2, DC),
        f"{pre}_AT": AT.astype(mf),
    }
    per_core = []
    x = np.asarray(x, f)
    nw = np.asarray(nw, f)
    nb = np.asarray(nb, f)
    for b in range(B):
        xT = np.zeros((D, S), f)
        xT[:, :Sr] = x[b].T
        nwT = np.zeros((D, S), f)
        nwT[:, :Sr] = nw[b].T
        nbT = np.zeros((D, S), f)
        nbT[:, :Sr] = nb[b].T
        m = {f"{pre}_xT": xT.astype(mf), f"{pre}_nwT": nwT, f"{pre}_nbT": nbT}
        if MM_DTYPE == "bf16":
            m[f"{pre}_xR"] = xT
        per_core.append(m)
    return shared, per_core


def kernel(**inputs):
    nc = _get_compiled()

    sh_t, pc_t = _prep_branch_inputs(
        "t",
        inputs["text_feature"],
        inputs["t_Wq"], inputs["t_bq"], inputs["t_Wk"], inputs["t_bk"],
        inputs["t_Wv"], inputs["t_bv"], inputs["t_W1"], inputs["t_b1"],
        inputs["t_W2"], inputs["t_b2"], inputs["t_nw"], inputs["t_nb"],
        TEXT,
    )
    sh_i, pc_i = _prep_branch_inputs(
        "i",
        inputs["image_feature"],
        inputs["i_Wq"], inputs["i_bq"], inputs["i_Wk"], inputs["i_bk"],
        inputs["i_Wv"], inputs["i_bv"], inputs["i_W1"], inputs["i_b1"],
        inputs["i_W2"], inputs["i_b2"], inputs["i_nw"], inputs["i_nb"],
        IMG,
    )

    in_maps = []
    for c in range(N_CORES):
        m = {}
        m.update(sh_t)
        m.update(sh_i)
        m.update(pc_t[c])
        m.update(pc_i[c])
        in_maps.append(m)

    global LAST_RESULT
    res = run_bass_kernel_spmd(
        nc, in_maps, core_ids=list(range(N_CORES)), trace=TRACE, **TRACE_KWARGS
    )
    LAST_RESULT = res
    outs = res.results

    f = np.float32
    text_out = np.stack(
        [np.asarray(outs[c]["t_outT"], f)[:, : TEXT["Sr"]].T for c in range(B)]
    )
    image_out = np.stack(
        [np.asarray(outs[c]["i_outT"], f)[:, : IMG["Sr"]].T for c in range(B)]
    )
    return (text_out, image_out)


if __name__ == "__main__":
    nc = build_kernel()
    counts = {}
    for bb in nc.main_func.blocks:
        for ins in bb.instructions:
            counts[type(ins).__name__] = counts.get(type(ins).__name__, 0) + 1
    print(counts)
